# revision 2
# baseline (speedup 1.0000x reference)
"""MoE expert-FFN kernel for Trainium2, expert-parallel across 8 NeuronCores.

Problem: out[t] = silu(x[t] @ W1[e_t]^T) @ W2[e_t]^T with
  E=64 experts, D=512, H=1024, T=256 tokens.

Strategy (memory-bound on expert weights, ~268MB fp32 total):
  - Core c owns experts [8c, 8c+8). Host routes tokens to the core owning
    their expert (all-to-all done on host since we hold full inputs).
  - Host pre-packs weights into the exact SBUF matmul layout:
      per expert slot s: [128 partitions, 4*1024 (W1^T) + 8*512 (W2^T)]
    so the device does nothing but stream 4MiB/expert with perfect DMAs.
  - On device per expert: H^T = W1^T-tiles.T @ x^T (PE, weights stationary),
    silu on ACT (PSUM->SBUF), Y^T = W2^T-tiles.T @ H^T, copy PSUM->SBUF,
    one gathered output DMA at the end.
  - Tokens are padded per expert to a fixed capacity C (static shapes).
"""

import numpy as np

E, D, H, T = 64, 512, 1024, 256
NCORES = 8
EPC = E // NCORES          # experts per core
DC = D // 128              # 4 d-chunks
HC = H // 128              # 8 h-chunks
WCOLS = DC * H + HC * D    # 8192 free columns of packed weights per expert

_prog_cache = {}


def _build_program(C, w_bufs=3):
    import concourse.bass as bass
    import concourse.mybir as mybir
    import concourse.tile as tile
    from concourse import bacc

    f32 = mybir.dt.float32
    nc = bacc.Bacc("TRN2", target_bir_lowering=False, debug=False)

    wts = nc.dram_tensor("wts", [EPC, 128, WCOLS], f32, kind="ExternalInput")
    xt = nc.dram_tensor("xt", [128, EPC * DC * C], f32, kind="ExternalInput")
    yt = nc.dram_tensor("yt", [128, EPC * DC * C], f32, kind="ExternalOutput")

    # PSUM budget: 8 banks of 512 fp32. Shrink bufs if C is large.
    ps1_banks = -(-HC * C // 512)
    ps2_banks = -(-DC * C // 512)
    ps1_bufs = 2 if 2 * ps1_banks + 2 * ps2_banks <= 8 else 1
    ps2_bufs = 2 if 2 * ps1_banks * ps1_bufs // 2 + 2 * ps2_banks <= 8 else 1

    with tile.TileContext(nc) as tc:
        with (
            tc.tile_pool(name="wpool", bufs=w_bufs) as wpool,
            tc.tile_pool(name="xpool", bufs=1) as xpool,
            tc.tile_pool(name="hpool", bufs=2) as hpool,
            tc.tile_pool(name="ypool", bufs=1) as ypool,
            tc.tile_pool(name="ps1", bufs=ps1_bufs, space="PSUM") as ps1p,
            tc.tile_pool(name="ps2", bufs=ps2_bufs, space="PSUM") as ps2p,
        ):
            xall = xpool.tile([128, EPC * DC * C], f32)
            nc.sync.dma_start(xall[:], xt[:])
            yall = ypool.tile([128, EPC * DC * C], f32)

            for s in range(EPC):
                w = wpool.tile([128, WCOLS], f32, tag="w")
                nc.sync.dma_start(w[:], wts[s])

                psh = ps1p.tile([128, HC * C], f32, tag="psh")
                for j in range(HC):
                    for c in range(DC):
                        nc.tensor.matmul(
                            psh[:, j * C:(j + 1) * C],
                            w[:, c * H + j * 128: c * H + j * 128 + 128],
                            xall[:, (s * DC + c) * C:(s * DC + c + 1) * C],
                            start=(c == 0),
                            stop=(c == DC - 1),
                        )

                sig = hpool.tile([128, HC * C], f32, tag="sig")
                nc.scalar.activation(
                    sig[:], psh[:], mybir.ActivationFunctionType.Sigmoid
                )
                hbuf = hpool.tile([128, HC * C], f32, tag="h")
                nc.vector.tensor_mul(hbuf[:], psh[:], sig[:])

                psy = ps2p.tile([128, DC * C], f32, tag="psy")
                for jd in range(DC):
                    for ch in range(HC):
                        nc.tensor.matmul(
                            psy[:, jd * C:(jd + 1) * C],
                            w[:, DC * H + ch * D + jd * 128:
                              DC * H + ch * D + jd * 128 + 128],
                            hbuf[:, ch * C:(ch + 1) * C],
                            start=(ch == 0),
                            stop=(ch == HC - 1),
                        )

                nc.vector.tensor_copy(
                    yall[:, s * DC * C:(s + 1) * DC * C], psy[:]
                )

            nc.sync.dma_start(yt[:], yall[:])

    nc.compile()
    return nc


def _route(expert_idx):
    idx = np.asarray(expert_idx).astype(np.int64)
    order = np.argsort(idx, kind="stable")
    counts = np.bincount(idx, minlength=E)
    starts = np.zeros(E + 1, dtype=np.int64)
    starts[1:] = np.cumsum(counts)
    return order, starts, counts


def _pack_inputs(x, fc1_w, fc2_w, order, starts, C):
    in_maps = []
    for core in range(NCORES):
        wh = np.empty((EPC, 128, WCOLS), np.float32)
        xh = np.zeros((128, EPC * DC * C), np.float32)
        for s in range(EPC):
            e = core * EPC + s
            # W1^T = fc1_w[e].T : [D, H]; d = c*128 + p -> col c*H + h
            w1t = np.ascontiguousarray(fc1_w[e].T).reshape(DC, 128, H)
            wh[s, :, :DC * H] = w1t.transpose(1, 0, 2).reshape(128, DC * H)
            # W2^T = fc2_w[e].T : [H, D]; h = ch*128 + p -> col DC*H + ch*D + d
            w2t = np.ascontiguousarray(fc2_w[e].T).reshape(HC, 128, D)
            wh[s, :, DC * H:] = w2t.transpose(1, 0, 2).reshape(128, HC * D)

            toks = order[starts[e]:starts[e + 1]]
            n = len(toks)
            if n:
                xte = np.ascontiguousarray(x[toks].T).reshape(DC, 128, n)
                for c in range(DC):
                    base = (s * DC + c) * C
                    xh[:, base:base + n] = xte[c]
        in_maps.append({"wts": wh, "xt": xh})
    return in_maps


def _unpack_outputs(results, order, starts, C, out_dtype):
    out = np.zeros((T, D), out_dtype)
    for core in range(NCORES):
        yh = np.asarray(results[core]["yt"])
        for s in range(EPC):
            e = core * EPC + s
            toks = order[starts[e]:starts[e + 1]]
            n = len(toks)
            if n == 0:
                continue
            blk = yh[:, s * DC * C:(s + 1) * DC * C].reshape(128, DC, C)
            out[toks] = (
                blk[:, :, :n].transpose(2, 1, 0).reshape(n, D)
            )
    return out


def kernel(x, expert_idx, fc1_w, fc2_w):
    from concourse.bass_utils import run_bass_kernel_spmd

    x = np.asarray(x, dtype=np.float32)
    fc1_w = np.asarray(fc1_w, dtype=np.float32)
    fc2_w = np.asarray(fc2_w, dtype=np.float32)

    order, starts, counts = _route(expert_idx)
    C = max(4, int(-(-int(counts.max()) // 4) * 4))

    if C not in _prog_cache:
        _prog_cache[C] = _build_program(C)
    nc = _prog_cache[C]

    in_maps = _pack_inputs(x, fc1_w, fc2_w, order, starts, C)
    res = run_bass_kernel_spmd(nc, in_maps, list(range(NCORES)))
    return _unpack_outputs(res.results, order, starts, C, np.float32)
